# revision 2
# baseline (speedup 1.0000x reference)
"""Trainium2 Bass kernel for nn_CrossAttentionFusion.

Math (per batch row, after host-side weight folding in float64):
  s_g  = [rowsum(Hg * T1) + Hg.a1 + Hr.b1 + c1]          T1 = Hr @ M1
  s_r  = [rowsum(Hr * T2) + Hr.a2 + Hg.b2 + c2]          T2 = Hg @ M2
  wg, wr = sigmoid(s_g), sigmoid(s_r)
  out_g = Hg + wg * (U1 + c_r2g)                         U1 = Hr @ A1
  out_r = Hr + wr * (U2 + c_g2r)                         U2 = Hg @ A2
  h_fused = [out_g, out_r];  imp = [sigmoid(wg-wr), sigmoid(wr-wg)]

where (all scaled by 1/sqrt(d) where appropriate):
  M1 = Wk_rnn^T Wq_gnn / sqrt(512)    (1024, 512)
  M2 = Wk_gnn^T Wq_rnn / sqrt(1024)   (512, 1024)
  A1 = (W_r2g Wv_rnn)^T               (1024, 512)
  A2 = (W_g2r Wv_gnn)^T               (512, 1024)

This replaces the reference's 8 matmuls (4.72M MAC/row) with 4 (2.10M MAC/row).

Device layout: pure data parallel over 8 cores (8192 rows each); per core,
64 batch tiles of 128 rows. Activations live natural (batch on partitions);
H^T chunks for matmul stationary are produced on-chip with PE fp32
transposes, rounded to float32r (TF32) by ACT copies. All 4 matmuls run in
float32r (1 cycle/row).
"""
import os
import sys

for _p in ("/opt/trn_rl_repo", "/root/.axon_site/_ro/trn_rl_repo"):
    if os.path.isdir(_p) and _p not in sys.path:
        sys.path.insert(0, _p)

import numpy as np  # noqa: E402

N_CORES = 8
B = 65536
BC = B // N_CORES          # rows per core
DG, DR = 512, 1024
PT = 128                   # batch rows per tile
NT_FULL = BC // PT         # 64 batch tiles per core

_cache = {}


def _fold_weights(Wq_gnn, bq_gnn, Wk_rnn, bk_rnn, Wq_rnn, bq_rnn, Wk_gnn, bk_gnn,
                  Wv_gnn, bv_gnn, Wv_rnn, bv_rnn, W_r2g, W_g2r):
    f8 = np.float64
    sg = 1.0 / np.sqrt(512.0)
    sr = 1.0 / np.sqrt(1024.0)
    Wq_gnn, Wk_rnn = np.asarray(Wq_gnn, f8), np.asarray(Wk_rnn, f8)
    Wq_rnn, Wk_gnn = np.asarray(Wq_rnn, f8), np.asarray(Wk_gnn, f8)
    Wv_gnn, Wv_rnn = np.asarray(Wv_gnn, f8), np.asarray(Wv_rnn, f8)
    W_r2g, W_g2r = np.asarray(W_r2g, f8), np.asarray(W_g2r, f8)
    bq_gnn, bk_rnn = np.asarray(bq_gnn, f8), np.asarray(bk_rnn, f8)
    bq_rnn, bk_gnn = np.asarray(bq_rnn, f8), np.asarray(bk_gnn, f8)
    bv_gnn, bv_rnn = np.asarray(bv_gnn, f8), np.asarray(bv_rnn, f8)

    out = {}
    out["m1"] = (Wk_rnn.T @ Wq_gnn) * sg          # (1024, 512): T1 = Hr @ m1
    out["a1"] = (Wq_gnn.T @ bk_rnn) * sg          # (512,)  pairs with Hg
    out["b1"] = (Wk_rnn.T @ bq_gnn) * sg          # (1024,) pairs with Hr
    out["c1"] = float((bq_gnn @ bk_rnn) * sg)
    out["m2"] = (Wk_gnn.T @ Wq_rnn) * sr          # (512, 1024): T2 = Hg @ m2
    out["a2"] = (Wq_rnn.T @ bk_gnn) * sr          # (1024,) pairs with Hr
    out["b2"] = (Wk_gnn.T @ bq_rnn) * sr          # (512,)  pairs with Hg
    out["c2"] = float((bq_rnn @ bk_gnn) * sr)
    out["A1"] = (W_r2g @ Wv_rnn).T                # (1024, 512): U1 = Hr @ A1
    out["vb1"] = W_r2g @ bv_rnn                   # (512,)
    out["A2"] = (W_g2r @ Wv_gnn).T                # (512, 1024): U2 = Hg @ A2
    out["vb2"] = W_g2r @ bv_gnn                   # (1024,)
    return {k: (np.float32(v) if isinstance(v, float) else v.astype(np.float32))
            for k, v in out.items()}


def _build_program(nt=NT_FULL, repeat=1):
    import concourse.tile as tile
    from concourse import mybir, bacc
    from concourse.masks import make_identity

    f32, f32r = mybir.dt.float32, mybir.dt.float32r
    mult, add, sub = (mybir.AluOpType.mult, mybir.AluOpType.add,
                      mybir.AluOpType.subtract)
    X = mybir.AxisListType.X
    SIG = mybir.ActivationFunctionType.Sigmoid
    bc_rows = nt * PT

    nc = bacc.Bacc("TRN2", debug=False, num_devices=N_CORES)
    hg = nc.dram_tensor("hg", (bc_rows, DG), f32, kind="ExternalInput").ap()
    hr = nc.dram_tensor("hr", (bc_rows, DR), f32, kind="ExternalInput").ap()
    m1w = nc.dram_tensor("m1w", (DR, DG), f32r, kind="ExternalInput").ap()
    m2w = nc.dram_tensor("m2w", (DG, DR), f32r, kind="ExternalInput").ap()
    a1w = nc.dram_tensor("a1w", (DR, DG), f32r, kind="ExternalInput").ap()
    a2w = nc.dram_tensor("a2w", (DG, DR), f32r, kind="ExternalInput").ap()
    rowin = {}
    for name, d in (("a1r", DG), ("b1r", DR), ("b2r", DG), ("a2r", DR),
                    ("vb1r", DG), ("vb2r", DR), ("c1r", 1), ("c2r", 1)):
        rowin[name] = nc.dram_tensor(name, (1, d), f32, kind="ExternalInput").ap()
    fused = nc.dram_tensor("fused", (bc_rows, DG + DR), f32,
                           kind="ExternalOutput").ap()
    imp = nc.dram_tensor("imp", (bc_rows, 2), f32, kind="ExternalOutput").ap()

    def bcast(ap, parts=128):
        import concourse.bass as bass
        return bass.AP(tensor=ap.tensor, offset=ap.offset,
                       ap=[[0, parts]] + ap.ap[1:])

    with tile.TileContext(nc) as tc:
        with tc.tile_pool(name="const", bufs=1) as cp, \
             tc.tile_pool(name="io", bufs=3) as iop, \
             tc.tile_pool(name="ht", bufs=2) as htp, \
             tc.tile_pool(name="scr", bufs=2) as scrp, \
             tc.tile_pool(name="small", bufs=4) as smp, \
             tc.tile_pool(name="pst", bufs=2, space="PSUM") as pst, \
             tc.tile_pool(name="psmm", bufs=1, space="PSUM") as psmm:

            ident = cp.tile([128, 128], f32)
            make_identity(nc, ident[:])
            m1t = cp.tile([128, DR // 128, DG], f32r)
            nc.sync.dma_start(m1t[:], m1w.rearrange("(k p) n -> p k n", p=128))
            a1t = cp.tile([128, DR // 128, DG], f32r)
            nc.sync.dma_start(a1t[:], a1w.rearrange("(k p) n -> p k n", p=128))
            m2t = cp.tile([128, DG // 128, DR], f32r)
            nc.sync.dma_start(m2t[:], m2w.rearrange("(k p) n -> p k n", p=128))
            a2t = cp.tile([128, DG // 128, DR], f32r)
            nc.sync.dma_start(a2t[:], a2w.rearrange("(k p) n -> p k n", p=128))

            rows = {}
            for name, d in (("a1r", DG), ("b1r", DR), ("b2r", DG),
                            ("a2r", DR), ("vb1r", DG), ("vb2r", DR),
                            ("c1r", 1), ("c2r", 1)):
                tl = cp.tile([128, d], f32, tag=name)
                nc.sync.dma_start(tl[:], bcast(rowin[name][:, :]))
                rows[name] = tl
            zerob = cp.tile([128, 1], f32)
            nc.vector.memset(zerob[:], 0.0)

            for rp in range(repeat):
                for t in range(nt):
                    r0 = t * PT
                    hgt = iop.tile([PT, DG], f32, tag="hg")
                    nc.sync.dma_start(hgt[:], hg[r0:r0 + PT, :])
                    hrt = iop.tile([PT, DR], f32, tag="hr")
                    nc.sync.dma_start(hrt[:], hr[r0:r0 + PT, :])

                    # --- transposes: natural H -> f32r H^T chunks ---
                    hgT = htp.tile([128, DG], f32r, tag="hgT")
                    trp = pst.tile([128, 512], f32, tag="tr")
                    for k in range(4):
                        nc.tensor.transpose(trp[:, k * 128:(k + 1) * 128],
                                            hgt[:, k * 128:(k + 1) * 128],
                                            ident[:])
                    nc.scalar.copy(hgT[:], trp[:])
                    hrT = htp.tile([128, DR], f32r, tag="hrT")
                    for h in range(2):
                        trp = pst.tile([128, 512], f32, tag="tr")
                        for k in range(4):
                            kk = h * 4 + k
                            nc.tensor.transpose(trp[:, k * 128:(k + 1) * 128],
                                                hrt[:, kk * 128:(kk + 1) * 128],
                                                ident[:])
                        nc.scalar.copy(hrT[:, h * 512:(h + 1) * 512], trp[:])

                    # --- main f32r matmuls ---
                    t1p = psmm.tile([128, DG], f32, tag="t1")
                    for k in range(8):
                        nc.tensor.matmul(t1p[:], lhsT=hrT[:, k * 128:(k + 1) * 128],
                                         rhs=m1t[:, k, :], start=(k == 0),
                                         stop=(k == 7))
                    u1p = psmm.tile([128, DG], f32, tag="u1")
                    for k in range(8):
                        nc.tensor.matmul(u1p[:], lhsT=hrT[:, k * 128:(k + 1) * 128],
                                         rhs=a1t[:, k, :], start=(k == 0),
                                         stop=(k == 7))
                    t2p = psmm.tile([128, DR], f32, tag="t2")
                    for h in range(2):
                        for k in range(4):
                            nc.tensor.matmul(t2p[:, h * 512:(h + 1) * 512],
                                             lhsT=hgT[:, k * 128:(k + 1) * 128],
                                             rhs=m2t[:, k, h * 512:(h + 1) * 512],
                                             start=(k == 0), stop=(k == 3))
                    u2p = psmm.tile([128, DR], f32, tag="u2")
                    for h in range(2):
                        for k in range(4):
                            nc.tensor.matmul(u2p[:, h * 512:(h + 1) * 512],
                                             lhsT=hgT[:, k * 128:(k + 1) * 128],
                                             rhs=a2t[:, k, h * 512:(h + 1) * 512],
                                             start=(k == 0), stop=(k == 3))

                    # --- scores ---
                    accg = smp.tile([128, 3], f32, tag="accg")
                    s1 = scrp.tile([128, DG], f32, tag="s512")
                    nc.vector.scalar_tensor_tensor(
                        out=s1[:], in0=t1p[:], scalar=1.0, in1=hgt[:],
                        op0=mult, op1=mult, accum_out=accg[:, 0:1])
                    s2 = scrp.tile([128, DG], f32, tag="s512")
                    nc.vector.scalar_tensor_tensor(
                        out=s2[:], in0=rows["a1r"][:], scalar=1.0, in1=hgt[:],
                        op0=mult, op1=mult, accum_out=accg[:, 1:2])
                    s3 = scrp.tile([128, DR], f32, tag="s1024")
                    nc.vector.scalar_tensor_tensor(
                        out=s3[:], in0=rows["b1r"][:], scalar=1.0, in1=hrt[:],
                        op0=mult, op1=mult, accum_out=accg[:, 2:3])
                    sg_ = smp.tile([128, 1], f32, tag="sg")
                    nc.vector.tensor_reduce(out=sg_[:], in_=accg[:, 0:3],
                                            axis=X, op=add)
                    wg = smp.tile([128, 1], f32, tag="wg")
                    nc.scalar.activation(wg[:], sg_[:], SIG,
                                         bias=rows["c1r"][:], scale=1.0)

                    accr = smp.tile([128, 3], f32, tag="accr")
                    s4 = scrp.tile([128, DR], f32, tag="s1024")
                    nc.vector.scalar_tensor_tensor(
                        out=s4[:], in0=t2p[:], scalar=1.0, in1=hrt[:],
                        op0=mult, op1=mult, accum_out=accr[:, 0:1])
                    s5 = scrp.tile([128, DR], f32, tag="s1024")
                    nc.vector.scalar_tensor_tensor(
                        out=s5[:], in0=rows["a2r"][:], scalar=1.0, in1=hrt[:],
                        op0=mult, op1=mult, accum_out=accr[:, 1:2])
                    s6 = scrp.tile([128, DG], f32, tag="s512")
                    nc.vector.scalar_tensor_tensor(
                        out=s6[:], in0=rows["b2r"][:], scalar=1.0, in1=hgt[:],
                        op0=mult, op1=mult, accum_out=accr[:, 2:3])
                    sr_ = smp.tile([128, 1], f32, tag="sr")
                    nc.vector.tensor_reduce(out=sr_[:], in_=accr[:, 0:3],
                                            axis=X, op=add)
                    wr = smp.tile([128, 1], f32, tag="wr")
                    nc.scalar.activation(wr[:], sr_[:], SIG,
                                         bias=rows["c2r"][:], scale=1.0)

                    # --- gated residual outputs ---
                    tmpg = scrp.tile([128, DG], f32, tag="tmpg")
                    nc.vector.scalar_tensor_tensor(
                        out=tmpg[:], in0=rows["vb1r"][:], scalar=wg[:],
                        in1=hgt[:], op0=mult, op1=add)
                    og = iop.tile([128, DG], f32, tag="og")
                    nc.vector.scalar_tensor_tensor(
                        out=og[:], in0=u1p[:], scalar=wg[:], in1=tmpg[:],
                        op0=mult, op1=add)
                    nc.sync.dma_start(fused[r0:r0 + PT, 0:DG], og[:])

                    tmpr = scrp.tile([128, DR], f32, tag="tmpr")
                    nc.vector.scalar_tensor_tensor(
                        out=tmpr[:], in0=rows["vb2r"][:], scalar=wr[:],
                        in1=hrt[:], op0=mult, op1=add)
                    orr = iop.tile([128, DR], f32, tag="orr")
                    nc.vector.scalar_tensor_tensor(
                        out=orr[:], in0=u2p[:], scalar=wr[:], in1=tmpr[:],
                        op0=mult, op1=add)
                    nc.sync.dma_start(fused[r0:r0 + PT, DG:], orr[:])

                    # --- importance ---
                    dwt = smp.tile([128, 1], f32, tag="dwt")
                    nc.vector.tensor_tensor(out=dwt[:], in0=wg[:], in1=wr[:],
                                            op=sub)
                    impt = smp.tile([128, 2], f32, tag="imp")
                    nc.scalar.activation(impt[:, 0:1], dwt[:], SIG,
                                         bias=zerob[:], scale=1.0)
                    nc.scalar.activation(impt[:, 1:2], dwt[:], SIG,
                                         bias=zerob[:], scale=-1.0)
                    nc.sync.dma_start(imp[r0:r0 + PT, :], impt[:])

    nc.compile()
    return nc


def _get_program(nt=NT_FULL, repeat=1):
    key = (nt, repeat)
    if key not in _cache:
        _cache[key] = _build_program(nt, repeat)
    return _cache[key]


def _make_in_maps(h_gnn, h_rnn, fw, nt=NT_FULL):
    rows = nt * PT
    h_gnn = np.ascontiguousarray(np.asarray(h_gnn, np.float32))
    h_rnn = np.ascontiguousarray(np.asarray(h_rnn, np.float32))
    shared = {
        "m1w": fw["m1"], "m2w": fw["m2"], "a1w": fw["A1"], "a2w": fw["A2"],
        "a1r": fw["a1"][None, :], "b1r": fw["b1"][None, :],
        "b2r": fw["b2"][None, :], "a2r": fw["a2"][None, :],
        "vb1r": fw["vb1"][None, :], "vb2r": fw["vb2"][None, :],
        "c1r": np.array([[fw["c1"]]], np.float32),
        "c2r": np.array([[fw["c2"]]], np.float32),
    }
    shared = {k: np.ascontiguousarray(v) for k, v in shared.items()}
    in_maps = []
    for c in range(N_CORES):
        m = dict(shared)
        m["hg"] = h_gnn[c * rows:(c + 1) * rows]
        m["hr"] = h_rnn[c * rows:(c + 1) * rows]
        in_maps.append(m)
    return in_maps


def kernel(h_gnn, h_rnn, Wq_gnn, bq_gnn, Wk_rnn, bk_rnn, Wq_rnn, bq_rnn,
           Wk_gnn, bk_gnn, Wv_gnn, bv_gnn, Wv_rnn, bv_rnn, W_r2g, W_g2r):
    from concourse.bass_utils import run_bass_kernel_spmd

    fw = _fold_weights(Wq_gnn, bq_gnn, Wk_rnn, bk_rnn, Wq_rnn, bq_rnn,
                       Wk_gnn, bk_gnn, Wv_gnn, bv_gnn, Wv_rnn, bv_rnn,
                       W_r2g, W_g2r)
    nc = _get_program()
    in_maps = _make_in_maps(h_gnn, h_rnn, fw)
    res = run_bass_kernel_spmd(nc, in_maps, core_ids=list(range(N_CORES)))
    h_fused = np.concatenate([r["fused"] for r in res.results], axis=0)
    imp = np.concatenate([r["imp"] for r in res.results], axis=0)
    return h_fused, imp
